# revision 28
# baseline (speedup 1.0000x reference)
"""DeepSeekV3 MoE router on 8 TRN2 NeuronCores (Bass/Tile).

Strategy (hardcoded for T=8192, D=7168, E=256, top-k=8, 8 groups, top-4 groups):
  - Data-parallel over tokens: each of 8 cores handles 1024 tokens.
  - Router weight kernel_DE and bias replicated to every core.
  - Matmul runs as a scaled fp16 3-term split (3xTF32-style):
        S*z = xh.(S*wh) + xh.(S*wl) + xl.(S*wh),   S = 2^11,
    where xh=fp16(x), xl=fp16(x-xh) (unscaled: it only ever multiplies
    the small S*wh weights, so its fp16-subnormal quantization error is
    negligible), whs=S*wh (exact fp16 exponent shift), and
    wls=fp16((w-wh)*S) (the scaling keeps the w-residual out of fp16
    subnormal range, where it would otherwise lose ~8 bits). All three
    terms carry the same factor S, so they accumulate into ONE PSUM
    bank and the sigmoid reads it with scale=-1/S (exact). fp16 passes
    run at 1 PE cycle/row vs native fp32's 4, so z error is fp32-class
    (~3e-7 std) while the matmul costs 3 cycles/row.
  - Host pre-arranges the pieces in lhsT chunk layout (contraction dim
    D on partitions), so no on-chip transposes.
  - Warm-up: dummy matmuls during the initial DMA fill pre-ramp the
    PE HAM clock gate (0.65/1.2 -> 2.4 GHz) before real work arrives.
  - Tiles 0-2 are processed interleaved per W-quarter so the 7.3MB
    W stream overlaps three tiles' compute (PE 13.5us/quarter-trio vs
    DMA 11.2us) instead of starving tile 0; tiles 3-7 run sequentially
    and their DVE chains pipeline under the matmuls. Chunk order within
    every tile stays ascending, so PSUM accumulation order (and thus
    the exact z bits) is independent of this scheduling.
  - Per 128-token tile: 3*56 accumulating fp16 matmuls -> PSUM,
    sigmoid via ACT Exp + DVE recip (matches XLA logistic lowering
    bit-for-bit), grouped top-2 / top-4-groups / top-8 with DVE Max8 /
    max_index / match_replace ops, normalize, DMA out.
"""

import sys

for p in ("/opt/trn_rl_repo", "/root/.axon_site/_ro/trn_rl_repo"):
    if p not in sys.path:
        sys.path.insert(0, p)

import numpy as np

T = 8192
D = 7168
E = 256
TOP_K = 8
N_GROUPS = 8
EPG = E // N_GROUPS  # experts per group = 32
TOPK_GROUPS = 4
SCALE = 2.5
N_CORES = 8
TPC = T // N_CORES  # tokens per core = 1024
N_TILES = TPC // 128  # 8 token tiles per core
KC = D // 128  # 56 contraction chunks
KQ = 14  # chunks per W quarter
N_Q = KC // KQ  # 4 quarters
S_LO = 2048.0  # 2^11 residual piece scale
N_WARM = 28  # dummy matmuls to pre-ramp the PE clock
N_ILV = 3  # leading tiles interleaved per W-quarter (phase A)

_CACHE = {}


def _build_nc():
    import concourse.bacc as bacc
    import concourse.mybir as mybir
    import concourse.tile as tile

    f32 = mybir.dt.float32
    f16 = mybir.dt.float16
    u32 = mybir.dt.uint32
    X = mybir.AxisListType.X
    Alu = mybir.AluOpType

    nc = bacc.Bacc(trn_type="TRN2")
    xh_d = nc.declare_dram_parameter("xh", [128, N_TILES, KC, 128], f16, isOutput=False)
    xl_d = nc.declare_dram_parameter("xl", [128, N_TILES, KC, 128], f16, isOutput=False)
    whs_d = nc.declare_dram_parameter("whs", [128, KC, E], f16, isOutput=False)
    wls_d = nc.declare_dram_parameter("wls", [128, KC, E], f16, isOutput=False)
    b_d = nc.declare_dram_parameter("bias", [128, E], f32, isOutput=False)
    ow_d = nc.declare_dram_parameter("out_w", [N_TILES, 128, TOP_K], f32, isOutput=True)
    oi_d = nc.declare_dram_parameter("out_idx", [N_TILES, 128, TOP_K], u32, isOutput=True)

    with tile.TileContext(nc) as tc:
        with (
            tc.tile_pool(name="const", bufs=1) as cpool,
            tc.tile_pool(name="xin", bufs=2) as xpool,
            tc.tile_pool(name="xfirst", bufs=2) as xfpool,
            tc.tile_pool(name="work", bufs=2) as wpool,
            tc.tile_pool(name="small", bufs=2) as spool,
            tc.tile_pool(name="psumA", bufs=1, space="PSUM") as papool,
            tc.tile_pool(name="psum", bufs=2, space="PSUM") as ppool,
            tc.tile_pool(name="warmps", bufs=1, space="PSUM") as warmpool,
        ):
            # Phase A data: W pieces interleaved with the leading tiles' x
            # pieces, in PE consumption order. Two W streams: whs (S*wh)
            # and wls (S*(w-wh)). The W/bias streams go out on the ACT
            # HWDGE ring and x on the SP ring, so the two queues fill SBUF
            # in parallel (halves the critical first-piece latency and
            # removes head-of-line blocking between W and x). The first
            # piece is split [7,7] so the PE can start sooner; x piece
            # tiles cycle through a bufs=2 pool.
            A_PIECES = [7, 7, 14, 14, 14]  # sums to KC
            ak0 = [sum(A_PIECES[:i]) for i in range(len(A_PIECES))]
            whs_gs, wls_gs = [], []
            xA = {t: ([], []) for t in range(N_ILV)}  # tile -> (xh, xl) pieces
            bias_sb = cpool.tile([128, E], f32)
            for g, pk in enumerate(A_PIECES):
                k0 = ak0[g]
                # alternate whole piece-groups across the two rings so each
                # carries ~half the bytes (a per-stream split unbalances
                # pacing: the laggard ring starves the PE and the idle
                # re-engages the HAM throttle)
                ring = nc.scalar if g % 2 == 0 else nc.sync
                whs_g = cpool.tile([128, pk, E], f16, tag=f"whs{g}", name=f"whs{g}")
                wls_g = cpool.tile([128, pk, E], f16, tag=f"wls{g}", name=f"wls{g}")
                ring.dma_start(whs_g[:], whs_d[:, k0 : k0 + pk])
                ring.dma_start(wls_g[:], wls_d[:, k0 : k0 + pk])
                whs_gs.append(whs_g)
                wls_gs.append(wls_g)
                for t in range(N_ILV):
                    xh_p = xfpool.tile(
                        [128, pk, 128], f16, tag=f"xA{t}h", name=f"xA{t}hg{g}"
                    )
                    xl_p = xfpool.tile(
                        [128, pk, 128], f16, tag=f"xA{t}l", name=f"xA{t}lg{g}"
                    )
                    ring.dma_start(xh_p[:], xh_d[:, t, k0 : k0 + pk])
                    ring.dma_start(xl_p[:], xl_d[:, t, k0 : k0 + pk])
                    xA[t][0].append(xh_p)
                    xA[t][1].append(xl_p)
                if g == 0:
                    # bias is only needed by the first chain (~50us in);
                    # keep it off the critical first-piece ring
                    nc.sync.dma_start(bias_sb[:], b_d[:])

            # PE clock-gate warm-up: dummy matmuls on zeroed SBUF while the
            # first real DMA pieces are still in flight.
            warm_l = cpool.tile([128, 128], f16, tag="warml", name="warml")
            warm_r = cpool.tile([128, E], f16, tag="warmr", name="warmr")
            nc.vector.memset(warm_l[:], 0.0)
            nc.vector.memset(warm_r[:], 0.0)
            warm_ps = warmpool.tile([128, E], f32, tag="warmps")
            for _ in range(N_WARM):
                nc.tensor.matmul(
                    warm_ps[:], lhsT=warm_l[:], rhs=warm_r[:], start=True, stop=True
                )

            def wpiece_of(k):
                for i in range(len(ak0) - 1, -1, -1):
                    if k >= ak0[i]:
                        return i, k - ak0[i]
                raise AssertionError

            def mm3(ps, xh_p, xl_p, xo, k):
                # three fp16 passes of chunk k into one S-scaled accumulator
                wi, wo = wpiece_of(k)
                nc.tensor.matmul(
                    ps[:],
                    lhsT=xh_p[:, xo],
                    rhs=whs_gs[wi][:, wo],
                    start=(k == 0),
                    stop=False,
                )
                nc.tensor.matmul(
                    ps[:],
                    lhsT=xh_p[:, xo],
                    rhs=wls_gs[wi][:, wo],
                    start=False,
                    stop=False,
                )
                nc.tensor.matmul(
                    ps[:],
                    lhsT=xl_p[:, xo],
                    rhs=whs_gs[wi][:, wo],
                    start=False,
                    stop=(k == KC - 1),
                )

            def chain(ps, tt):
                """Post-matmul per-tile pipeline: sigmoid, grouped top-k,
                weight gather+normalize, output DMA."""
                # g = sigmoid(z) = 1/(1+exp(-z)), decomposed exactly as
                # XLA lowers logistic on this backend (bitwise-matching the
                # reference selection). ps holds S*z; scale=-1/S is an exact
                # power-of-two so ACT sees -z exactly.
                ex = wpool.tile([128, E], f32, tag="ex")
                nc.scalar.activation(
                    ex[:], ps[:], mybir.ActivationFunctionType.Exp, scale=-1.0 / S_LO
                )
                u = wpool.tile([128, E], f32, tag="u")
                nc.vector.tensor_scalar(u[:], ex[:], 1.0, None, op0=Alu.add)
                g = wpool.tile([128, E], f32, tag="g")
                nc.vector.reciprocal(g[:], u[:])
                s = wpool.tile([128, E], f32, tag="s")
                nc.vector.tensor_add(s[:], g[:], bias_sb[:])

                # grouped top-2 sums -> group scores [128, 8]
                s3 = s[:].rearrange("p (g e) -> p g e", g=N_GROUPS)
                m1 = spool.tile([128, N_GROUPS], f32, tag="m1")
                nc.vector.tensor_reduce(m1[:], s3, axis=X, op=Alu.max)
                s2 = wpool.tile([128, E], f32, tag="s2")
                nc.vector.match_replace(
                    out=s2[:], in_to_replace=m1[:], in_values=s[:], imm_value=-1e30
                )
                m2 = spool.tile([128, N_GROUPS], f32, tag="m2")
                nc.vector.tensor_reduce(
                    m2[:], s2[:].rearrange("p (g e) -> p g e", g=N_GROUPS), axis=X, op=Alu.max
                )
                gs = spool.tile([128, N_GROUPS], f32, tag="gs")
                nc.vector.tensor_add(gs[:], m1[:], m2[:])

                # top-4 groups: threshold = 4th largest group score
                g8 = spool.tile([128, 8], f32, tag="g8")
                nc.vector.max(g8[:], gs[:])
                gmask = spool.tile([128, N_GROUPS], f32, tag="gmask")
                nc.vector.tensor_scalar(
                    gmask[:], gs[:], g8[:, TOPK_GROUPS - 1 : TOPK_GROUPS], None, op0=Alu.is_ge
                )

                # s_sel = s * gmask (zeros outside selected groups)
                s_sel = wpool.tile([128, E], f32, tag="ssel")
                nc.vector.tensor_tensor(
                    s_sel[:].rearrange("p (g e) -> p g e", g=N_GROUPS),
                    s3,
                    gmask[:].to_broadcast([128, N_GROUPS, EPG]),
                    op=Alu.mult,
                )

                # top-8 experts by biased score
                top8 = spool.tile([128, 8], f32, tag="top8")
                nc.vector.max(top8[:], s_sel[:])
                idx = spool.tile([128, 8], u32, tag="idx")
                nc.vector.max_index(idx[:], top8[:], s_sel[:])

                # positions of the top-8 -> gather sigmoid values (unbiased):
                # z = (s_sel >= 8th_largest) * g in one fused op
                z = wpool.tile([128, E], f32, tag="z")
                nc.vector.scalar_tensor_tensor(
                    z[:], s_sel[:], top8[:, 7:8], g[:], op0=Alu.is_ge, op1=Alu.mult
                )
                z8 = spool.tile([128, 8], f32, tag="z8")
                nc.vector.max(z8[:], z[:])
                zidx = spool.tile([128, 8], u32, tag="zidx")
                nc.vector.max_index(zidx[:], z8[:], z[:])

                # align sigmoid values to the biased-score rank order:
                # w8[p, j] = sum_k (idx[p,j] == zidx[p,k]) * z8[p,k]
                idxf = spool.tile([128, 8], f32, tag="idxf")
                nc.vector.tensor_copy(idxf[:], idx[:])
                zidxf = spool.tile([128, 8], f32, tag="zidxf")
                nc.vector.tensor_copy(zidxf[:], zidx[:])
                eq = spool.tile([128, 8, 8], f32, tag="eq")
                nc.vector.tensor_tensor(
                    eq[:],
                    idxf[:].unsqueeze(2).broadcast_to([128, 8, 8]),
                    zidxf[:].unsqueeze(1).broadcast_to([128, 8, 8]),
                    op=Alu.is_equal,
                )
                wm = spool.tile([128, 8, 8], f32, tag="wm")
                nc.vector.tensor_tensor(
                    wm[:], eq[:], z8[:].unsqueeze(1).broadcast_to([128, 8, 8]), op=Alu.mult
                )
                w8 = spool.tile([128, 8], f32, tag="w8")
                nc.vector.tensor_reduce(w8[:], wm[:], axis=X, op=Alu.add)

                # normalize: out = w8 * (2.5 / (sum(w8) + 1e-20))
                den = spool.tile([128, 1], f32, tag="den")
                nc.vector.tensor_reduce(den[:], w8[:], axis=X, op=Alu.add)
                nc.vector.tensor_scalar(
                    den[:], den[:], 1e-20, 1.0 / SCALE, op0=Alu.add, op1=Alu.mult
                )
                rec = spool.tile([128, 1], f32, tag="rec")
                nc.vector.reciprocal(rec[:], den[:])
                wout = spool.tile([128, 8], f32, tag="wout")
                nc.vector.tensor_scalar(wout[:], w8[:], rec[:], None, op0=Alu.mult)

                nc.sync.dma_start(ow_d[tt], wout[:])
                nc.sync.dma_start(oi_d[tt], idx[:])

            # Phase A: leading tiles interleaved per W quarter (chunk order
            # within each tile stays ascending, so PSUM accumulation order is
            # unchanged vs sequential processing).
            psA = {
                t: papool.tile([128, E], f32, tag=f"psA{t}", name=f"psA{t}")
                for t in range(N_ILV)
            }
            for g, pk in enumerate(A_PIECES):
                for t in range(N_ILV):
                    for kk in range(pk):
                        k = ak0[g] + kk
                        mm3(psA[t], xA[t][0][g], xA[t][1][g], kk, k)
            for t in range(N_ILV):
                chain(psA[t], t)

            # Phase B: remaining tiles sequential; x streamed per quarter
            # with double-buffered pools, chains pipeline under the next
            # tile's matmuls.
            for tt in range(N_ILV, N_TILES):
                xh_ps, xl_ps = [], []
                ring = nc.scalar if tt % 2 == 0 else nc.sync
                for gq in range(N_Q):
                    k0 = gq * KQ
                    xh_p = xpool.tile([128, KQ, 128], f16, tag=f"xh{gq}", name=f"xhp{gq}")
                    ring.dma_start(xh_p[:], xh_d[:, tt, k0 : k0 + KQ])
                    xh_ps.append(xh_p)
                    xl_p = xpool.tile([128, KQ, 128], f16, tag=f"xl{gq}", name=f"xlp{gq}")
                    ring.dma_start(xl_p[:], xl_d[:, tt, k0 : k0 + KQ])
                    xl_ps.append(xl_p)

                ps = ppool.tile([128, E], f32, tag="ps")
                for gq in range(N_Q):
                    for kk in range(KQ):
                        k = gq * KQ + kk
                        mm3(ps, xh_ps[gq], xl_ps[gq], kk, k)
                chain(ps, tt)

    nc.finalize()
    return nc


def _get_nc():
    if "nc" not in _CACHE:
        _CACHE["nc"] = _build_nc()
    return _CACHE["nc"]


def _prep_inputs(x_TD, kernel_DE, bias_E):
    # fp16 hi/lo split (see module docstring)
    xh = x_TD.astype(np.float16)
    xl = (x_TD - xh.astype(np.float32)).astype(np.float16)
    wh = kernel_DE.astype(np.float16)
    whs = wh * np.float16(S_LO)  # exact exponent shift in fp16
    wls = ((kernel_DE - wh.astype(np.float32)) * np.float32(S_LO)).astype(np.float16)

    # w layout: w_sb[p, k, e] = w[k*128 + p, e]
    def wlay(w):
        return np.ascontiguousarray(w.reshape(KC, 128, E).transpose(1, 0, 2))

    whs_l, wls_l = wlay(whs), wlay(wls)
    bias_rep = np.ascontiguousarray(np.tile(bias_E[None, :], (128, 1)))
    in_maps = []
    for c in range(N_CORES):
        sl = slice(c * TPC, (c + 1) * TPC)
        # x_sb[p, tt, k, t] = xc[tt*128 + t, k*128 + p]
        xh_c = np.ascontiguousarray(
            xh[sl].reshape(N_TILES, 128, KC, 128).transpose(3, 0, 2, 1)
        )
        xl_c = np.ascontiguousarray(
            xl[sl].reshape(N_TILES, 128, KC, 128).transpose(3, 0, 2, 1)
        )
        in_maps.append(
            {"xh": xh_c, "xl": xl_c, "whs": whs_l, "wls": wls_l, "bias": bias_rep}
        )
    return in_maps


def kernel(x_TD, kernel_DE, bias_E, _trace=False):
    from concourse import bass_utils

    x_TD = np.asarray(x_TD, dtype=np.float32)
    kernel_DE = np.asarray(kernel_DE, dtype=np.float32)
    bias_E = np.asarray(bias_E, dtype=np.float32)

    nc = _get_nc()
    in_maps = _prep_inputs(x_TD, kernel_DE, bias_E)
    res = bass_utils.run_bass_kernel_spmd(
        nc, in_maps, core_ids=list(range(N_CORES)), trace=_trace
    )
    _CACHE["last_results"] = res
    weights = np.concatenate(
        [res.results[c]["out_w"].reshape(TPC, TOP_K) for c in range(N_CORES)], axis=0
    )
    indices = np.concatenate(
        [
            res.results[c]["out_idx"].reshape(TPC, TOP_K).astype(np.int32)
            for c in range(N_CORES)
        ],
        axis=0,
    )
    return weights, indices


if __name__ == "__main__":
    rng = np.random.default_rng(0)
    x = rng.standard_normal((T, D), dtype=np.float32)
    w = rng.standard_normal((D, E), dtype=np.float32) / np.sqrt(D)
    b = (rng.standard_normal(E) * 0.01).astype(np.float32)
    wts, idx = kernel(x, w, b)
    print("weights", wts.shape, wts.dtype, "indices", idx.shape, idx.dtype)
    print(wts[:2])
    print(idx[:2])
